# revision 33
# baseline (speedup 1.0000x reference)
# GAT layer kernel for Trainium2 (8 NeuronCores, SPMD).
#
# Reference computation (B=2, N=1024, C_IN=128, H=16, CPH=8):
#   e   = sigmoid(edges @ W_adj + b_adj);  e = (e + e^T)/2;  adj = e > 0.5
#   x   = node_feats @ W_proj + b_proj           -> [B,N,H,CPH]
#   lp  = sum_c x*a[:, :8];  lc = sum_c x*a[:, 8:]
#   L   = leaky_relu(lp[i,h] + lc[j,h], 0.2) masked by adj, softmax over j
#   out = einsum('bijh,bjhc->bihc', softmax, x)  -> [B,N,128]
#
# Sharding: core c handles batch b=c//4, parent rows i in chunk q=c%4 (256 rows).
#
# Key tricks:
#  * sigmoid is monotone => (sig(p)+sig(r))/2 > 0.5  <=>  p + r > 0.
#    So adj needs no sigmoid: adj[i,j] = (Zr[i,j]+Zr[j,i] > 0), Zr = edges@W_adj+b.
#  * Everything in the big [j, (h,i)] layout (j on partitions) so softmax sums
#    ride the PE einsum via a ones-column, and no on-device transposes of the
#    N^2 tensors are needed (host passes edges^T; both symmetrization terms
#    become natural-orientation matmuls).
#  * Softmax normalization applied to the *output* (divide by S after einsum).
#  * Per-core j-axis is rotated by q*256 so the SPMD program uses fixed slices.

import numpy as np

B, N, C_IN, H, CPH, C_OUT = 2, 1024, 128, 16, 8, 128
QN = N // 4          # rows per core
NKT = N // 128       # k tiles
NJT = N // 128       # j tiles
ALPHA = 0.2

_CACHE = {}
_DEBUG = False


def _build_bass():
    if "nc" in _CACHE:
        return _CACHE["nc"]
    from contextlib import ExitStack

    import concourse.mybir as mybir
    import concourse.tile as tile
    from concourse.bacc import Bacc

    fp32 = mybir.dt.float32
    bf16 = mybir.dt.bfloat16
    F = mybir.ActivationFunctionType
    ALU = mybir.AluOpType

    nc = Bacc()

    # DRAM I/O (per-core contents; j axis rotated by q*QN where noted)
    edgest_d = nc.dram_tensor("edgest", [N, N], fp32, kind="ExternalInput")   # edges[b].T, cols j-rotated
    wadjr_d = nc.dram_tensor("wadjr", [N, N], fp32, kind="ExternalInput")     # W_adj, cols j-rotated
    wchunk_d = nc.dram_tensor("wchunk", [N, QN], fp32, kind="ExternalInput")  # W_adj[:, chunk] (unrotated)
    echunk_d = nc.dram_tensor("echunk", [N, QN], fp32, kind="ExternalInput")  # edges[b].T[:, chunk]
    badjr_d = nc.dram_tensor("badjr", [1, N], fp32, kind="ExternalInput")     # b_adj, j-rotated
    badji_d = nc.dram_tensor("badji", [1, QN], fp32, kind="ExternalInput")    # b_adj[chunk]
    nft_d = nc.dram_tensor("nft", [C_IN, N], fp32, kind="ExternalInput")      # node_feats[b].T, cols j-rotated
    nftc_d = nc.dram_tensor("nftc", [C_IN, QN], fp32, kind="ExternalInput")   # node_feats[b,chunk].T
    wproj_d = nc.dram_tensor("wproj", [C_IN, C_OUT], fp32, kind="ExternalInput")
    bprojr_d = nc.dram_tensor("bprojr", [1, C_OUT], fp32, kind="ExternalInput")
    apm_d = nc.dram_tensor("apm", [C_OUT, H], fp32, kind="ExternalInput")     # block-diag a[:, :8]
    acm_d = nc.dram_tensor("acm", [C_OUT, H], fp32, kind="ExternalInput")     # block-diag a[:, 8:]
    ident_d = nc.dram_tensor("ident", [128, 128], fp32, kind="ExternalInput")
    rhsi_d = nc.dram_tensor("rhsi", [17, H * QN], fp32, kind="ExternalInput")
    lptmp_d = nc.dram_tensor("lptmp", [H * QN], fp32, kind="Internal")
    lctmp_d = nc.dram_tensor("lctmp", [17, N], fp32, kind="Internal")
    xmtmp_d = nc.dram_tensor("xmtmp", [C_OUT], fp32, kind="Internal")
    out_d = nc.dram_tensor("out", [QN, C_OUT], fp32, kind="ExternalOutput")
    if _DEBUG:
        dbg = {
            "d_m01": nc.dram_tensor("d_m01", [128, QN], fp32, kind="ExternalOutput"),
            "d_u": nc.dram_tensor("d_u", [128, H * QN], fp32, kind="ExternalOutput"),
            "d_lr": nc.dram_tensor("d_lr", [128, H * QN], fp32, kind="ExternalOutput"),
            "d_rhs17": nc.dram_tensor("d_rhs17", [17, H * QN], fp32, kind="ExternalOutput"),
            "d_lcones": nc.dram_tensor("d_lcones", [17, N], fp32, kind="ExternalOutput"),
            "d_xe0": nc.dram_tensor("d_xe0", [128, H * (CPH + 1)], fp32, kind="ExternalOutput"),
            "d_outts": nc.dram_tensor("d_outts", [2, 128, 512], fp32, kind="ExternalOutput"),
            "d_sis": nc.dram_tensor("d_sis", [2, 128, H], fp32, kind="ExternalOutput"),
        }

    with tile.TileContext(nc) as tc, ExitStack() as ctx:
        const = ctx.enter_context(tc.tile_pool(name="const", bufs=1))
        work = ctx.enter_context(tc.tile_pool(name="work", bufs=2))
        fin = ctx.enter_context(tc.tile_pool(name="fin", bufs=1))
        psq = ctx.enter_context(tc.tile_pool(name="psq", bufs=2, space="PSUM"))
        psz = ctx.enter_context(tc.tile_pool(name="psz", bufs=2, space="PSUM"))
        pse = ctx.enter_context(tc.tile_pool(name="pse", bufs=1, space="PSUM"))

        # ---------------- residents ----------------
        edt = const.tile([128, NKT, N], fp32, name="edt")      # edges^T (rot), [p, kt, j]
        wad = const.tile([128, NKT, N], fp32, name="wad")      # W_adj (rot cols)
        wch = const.tile([128, NKT, QN], fp32, name="wch")     # W_adj[:, chunk]
        ech = const.tile([128, NKT, QN], fp32, name="ech")     # edges^T[:, chunk]
        for kt in range(NKT):
            nc.sync.dma_start(edt[:, kt, :], edgest_d[kt * 128:(kt + 1) * 128, :])
            nc.sync.dma_start(wad[:, kt, :], wadjr_d[kt * 128:(kt + 1) * 128, :])
            nc.sync.dma_start(wch[:, kt, :], wchunk_d[kt * 128:(kt + 1) * 128, :])
            nc.sync.dma_start(ech[:, kt, :], echunk_d[kt * 128:(kt + 1) * 128, :])

        nft_s = const.tile([128, N], fp32, name="nft_s")
        nc.sync.dma_start(nft_s, nft_d[:, :])
        nftc_s = const.tile([128, QN], fp32, name="nftc_s")
        nc.sync.dma_start(nftc_s, nftc_d[:, :])
        wproj_s = const.tile([128, C_OUT], fp32, name="wproj_s")
        nc.sync.dma_start(wproj_s, wproj_d[:, :])
        bprojr_s = const.tile([1, C_OUT], fp32, name="bprojr_s")
        nc.sync.dma_start(bprojr_s, bprojr_d[:, :])
        apm_s = const.tile([128, H], fp32, name="apm_s")
        nc.sync.dma_start(apm_s, apm_d[:, :])
        acm_s = const.tile([128, H], fp32, name="acm_s")
        nc.sync.dma_start(acm_s, acm_d[:, :])
        badjr_s = const.tile([1, N], fp32, name="badjr_s")
        nc.sync.dma_start(badjr_s, badjr_d[:, :])
        badji_s = const.tile([1, QN], fp32, name="badji_s")
        nc.sync.dma_start(badji_s, badji_d[:, :])
        ident_s = const.tile([128, 128], fp32, name="ident_s")
        nc.sync.dma_start(ident_s, ident_d[:, :])

        # rhs17: row 0 = lp_flat (h,i); rows 1..16 = head indicator mask
        rhs17 = const.tile([17, H * QN], fp32, name="rhs17")
        nc.sync.dma_start(rhs17, rhsi_d[:, :])

        ones_row = const.tile([1, N], fp32, name="ones_row")
        nc.vector.memset(ones_row, 1.0)

        # ---------------- phase 0: projections ----------------
        # x^T [hc, n] for lc (rotated j space), full N
        xt_s = const.tile([128, N], fp32, name="xt_s")
        for half in range(2):
            ps = psq.tile([128, 512], fp32, tag="q")
            nc.tensor.matmul(ps, wproj_s, nft_s[:, half * 512:(half + 1) * 512],
                             start=True, stop=False)
            nc.tensor.matmul(ps, bprojr_s, ones_row[:, :512], start=False, stop=True)
            nc.scalar.copy(xt_s[:, half * 512:(half + 1) * 512], ps)

        # x^T for the i-chunk (unrotated) -> lp
        xtc_s = const.tile([128, QN], fp32, name="xtc_s")
        psc = psq.tile([128, 512], fp32, tag="q")
        nc.tensor.matmul(psc[:, :QN], wproj_s, nftc_s, start=True, stop=False)
        nc.tensor.matmul(psc[:, :QN], bprojr_s, ones_row[:, :QN], start=False, stop=True)
        nc.scalar.copy(xtc_s, psc[:, :QN])

        # lp [16, 256] -> rhs17 row 0 as (h,i)-flat
        lp_ps = psq.tile([128, 512], fp32, tag="q")
        nc.tensor.matmul(lp_ps[:H, :QN], apm_s, xtc_s, start=True, stop=True)
        lp_s = const.tile([H, QN], fp32, name="lp_s")
        nc.vector.tensor_copy(lp_s, lp_ps[:H, :QN])
        # [16,256] -> [1,4096] reshape via a DRAM bounce (DRAM is linear)
        nc.sync.dma_start(lptmp_d.rearrange("(h i) -> h i", h=H), lp_s)
        nc.sync.dma_start(rhs17[0:1, :], lptmp_d[None, :])

        # lc -> lcones rows 1..16; row 0 = ones. Partition-offset SBUF writes
        # are not allowed, so assemble the [17, N] tile in DRAM and DMA it in.
        lc_s = const.tile([H, N], fp32, name="lc_s")
        for half in range(2):
            lc_ps = psq.tile([128, 512], fp32, tag="q")
            nc.tensor.matmul(lc_ps[:H, :], acm_s, xt_s[:, half * 512:(half + 1) * 512],
                             start=True, stop=True)
            nc.vector.tensor_copy(lc_s[:, half * 512:(half + 1) * 512], lc_ps[:H, :])
        nc.sync.dma_start(lctmp_d[0:1, :], ones_row)
        nc.sync.dma_start(lctmp_d[1:17, :], lc_s)
        lcones = const.tile([17, N], fp32, name="lcones")
        nc.sync.dma_start(lcones, lctmp_d[:, :])

        # column means of x (uniform-attention fallback for all-masked rows):
        # reduce x^T over n, bounce through DRAM to a row, broadcast via PE.
        xsum_t = const.tile([128, 1], fp32, name="xsum_t")
        nc.vector.tensor_reduce(xsum_t, xt_s, mybir.AxisListType.X, ALU.add)
        nc.sync.dma_start(xmtmp_d.rearrange("(p o) -> p o", o=1), xsum_t)
        xmr_s = const.tile([1, C_OUT], fp32, name="xmr_s")
        nc.sync.dma_start(xmr_s, xmtmp_d[None, :])
        xm_ps = psq.tile([128, 512], fp32, tag="q")
        nc.tensor.matmul(xm_ps[:, :C_OUT], ones_row[:, :128], xmr_s,
                         start=True, stop=True)
        xmb = const.tile([128, H, CPH], fp32, name="xmb")
        nc.vector.tensor_scalar_mul(
            xmb.rearrange("p h c -> p (h c)"), xm_ps[:, :C_OUT], 1.0 / N)

        # x~ (einsum stationary): per j-tile [128j, 16h, 9] bf16, col 8 = 1.0
        xe = []
        for jt in range(NJT):
            ps = psq.tile([128, 512], fp32, tag="q")
            nc.tensor.matmul(ps[:, :128], nft_s[:, jt * 128:(jt + 1) * 128], wproj_s,
                             start=True, stop=False)
            nc.tensor.matmul(ps[:, :128], ones_row[:, :128], bprojr_s, start=False, stop=True)
            xej = const.tile([128, H, CPH + 1], bf16, name=f"xe{jt}")
            nc.vector.tensor_copy(
                xej[:, :, 0:CPH],
                ps[:, :128].rearrange("p (h c) -> p h c", c=CPH),
            )
            nc.vector.memset(xej[:, :, CPH:CPH + 1], 1.0)
            xe.append(xej)

        # einsum accumulators: head h -> tile t=h//8, free block gg=(h%8)//4,
        # partitions 32r..32r+9 with r=h%4 (row 32r+8 = softmax denom S)
        ab = [pse.tile([128, 512], fp32, name=f"ab{t}") for t in range(2)]

        # ---------------- main loop over j tiles ----------------
        for jt in range(NJT):
            # adjacency: Q[j,i] = Zr[j,i] + Zr[i,j] + b_adj[i] + b_adj[j]
            qps = psq.tile([128, 512], fp32, tag="q")
            q_ap = qps[:, :QN]
            for kt in range(NKT):
                nc.tensor.matmul(q_ap, edt[:, kt, jt * 128:(jt + 1) * 128],
                                 wch[:, kt, :], start=(kt == 0), stop=False)
                nc.tensor.matmul(q_ap, wad[:, kt, jt * 128:(jt + 1) * 128],
                                 ech[:, kt, :], start=False, stop=False)
            nc.tensor.matmul(q_ap, badjr_s[:, jt * 128:(jt + 1) * 128],
                             ones_row[:, :QN], start=False, stop=False)
            nc.tensor.matmul(q_ap, ones_row[:, :128], badji_s, start=False, stop=True)
            m01 = work.tile([128, QN], bf16, tag="m01")
            nc.vector.tensor_scalar(m01, q_ap, 0.0, None, ALU.is_gt)
            if _DEBUG and jt == 0:
                dm = fin.tile([128, QN], fp32, name="dm")
                nc.vector.tensor_copy(dm, m01)
                nc.sync.dma_start(dbg["d_m01"][:, :], dm)

            # logits Z[j,(h,i)] = lc[j,h] + lp[i,h] via K=17 matmul
            lr = work.tile([128, H * QN], fp32, tag="lr")
            for c4 in range(4):
                zps = psz.tile([128, 1024], fp32, tag="z")
                for half in range(2):
                    off = c4 * 1024 + half * 512
                    nc.tensor.matmul(zps[:, half * 512:(half + 1) * 512],
                                     lcones[:, jt * 128:(jt + 1) * 128],
                                     rhs17[:, off:off + 512], start=True, stop=True)
                nc.scalar.activation(lr[:, c4 * 1024:(c4 + 1) * 1024], zps,
                                     F.Prelu, alpha=ALPHA)

            u = work.tile([128, H, QN], bf16, tag="u")
            uflat = u.rearrange("p h i -> p (h i)")
            nc.scalar.activation(uflat, lr, F.Exp)
            nc.vector.tensor_tensor(
                u, u, m01[:, None, :].to_broadcast((128, H, QN)), ALU.mult)
            if _DEBUG and jt == 0:
                nc.sync.dma_start(dbg["d_lr"][:, :], lr)
                du = fin.tile([128, H * QN], fp32, name="du")
                nc.vector.tensor_copy(du, uflat)
                nc.sync.dma_start(dbg["d_u"][:, :], du)
                nc.sync.dma_start(dbg["d_rhs17"][:, :], rhs17)
                nc.sync.dma_start(dbg["d_lcones"][:, :], lcones)
                dx = fin.tile([128, H * (CPH + 1)], fp32, name="dx")
                nc.vector.tensor_copy(dx, xe[0].rearrange("p h c -> p (h c)"))
                nc.sync.dma_start(dbg["d_xe0"][:, :], dx)

            # einsum: out^T[(h,c),i] += x~_h^T @ U_h  (col 8 accumulates S)
            # start=True pends-zero the whole 2KB bank row for the touched
            # partitions, so only the first (gg=0) matmul per (t, r) row may
            # set it; gg=1's first write consumes the same pending-zero bytes.
            for h in range(H):
                t, sub = h // 8, h % 8
                gg, r = sub // 4, sub % 4
                nc.tensor.matmul(
                    ab[t][32 * r:32 * r + 9, gg * QN:(gg + 1) * QN],
                    xe[jt][:, h, :], u[:, h, :],
                    start=(jt == 0 and gg == 0), stop=(jt == NJT - 1),
                    tile_position=(0, 32 * r), skip_group_check=True)

        # ---------------- finalize: transpose, normalize, store ----------------
        outts = [fin.tile([128, 512], fp32, name=f"outts{t}") for t in range(2)]
        for t in range(2):
            nc.vector.memset(outts[t], 0.0)
            for r in range(4):
                eng = nc.scalar if (t + r) % 2 else nc.vector
                if eng is nc.scalar:
                    nc.scalar.copy(outts[t][32 * r:32 * r + 9, :],
                                   ab[t][32 * r:32 * r + 9, :])
                else:
                    nc.vector.tensor_copy(outts[t][32 * r:32 * r + 9, :],
                                          ab[t][32 * r:32 * r + 9, :])

        o_s = [fin.tile([128, H, CPH], fp32, name=f"os{ih}") for ih in range(2)]
        s_is = [fin.tile([128, H], fp32, name=f"sis{ih}") for ih in range(2)]
        for t in range(2):
            for gg in range(2):
                for ih in range(2):
                    pst = psq.tile([128, 512], fp32, tag="q")
                    nc.tensor.transpose(
                        pst[:, :128],
                        outts[t][:, gg * 256 + ih * 128: gg * 256 + (ih + 1) * 128],
                        ident_s)
                    p3 = pst[:, :128].rearrange("p (r c) -> p r c", c=32)
                    h0 = 8 * t + 4 * gg
                    nc.vector.tensor_copy(o_s[ih][:, h0:h0 + 4, :], p3[:, :, 0:CPH])
                    nc.vector.tensor_copy(s_is[ih][:, h0:h0 + 4, None], p3[:, :, CPH:CPH + 1])
        if _DEBUG:
            for t in range(2):
                nc.sync.dma_start(dbg["d_outts"][t, :, :], outts[t])
            for ih in range(2):
                nc.sync.dma_start(dbg["d_sis"][ih, :, :], s_is[ih])
        for ih in range(2):
            # guard against S=0 (all-masked row): reference softmax degrades to
            # uniform 1/N over all j there; patch those (i,h) with column means.
            s0 = fin.tile([128, H, CPH], mybir.dt.uint8, name=f"s0{ih}")
            nc.vector.tensor_scalar(
                s0, s_is[ih][:, :, None].to_broadcast((128, H, CPH)),
                0.0, None, ALU.is_equal)
            r_is = fin.tile([128, H], fp32, name=f"ris{ih}")
            nc.vector.tensor_scalar_max(r_is, s_is[ih], 1e-30)
            nc.vector.reciprocal(r_is, r_is)
            nc.vector.tensor_tensor(
                o_s[ih], o_s[ih], r_is[:, :, None].to_broadcast((128, H, CPH)),
                ALU.mult)
            nc.vector.copy_predicated(o_s[ih], s0, xmb)
            nc.sync.dma_start(
                out_d[ih * 128:(ih + 1) * 128, :],
                o_s[ih].rearrange("p h c -> p (h c)"))

    nc.finalize()
    _CACHE["nc"] = nc
    return nc


def _prep_in_maps(node_feats, edges, W_proj, b_proj, a, W_adj, b_adj):
    f32 = np.float32
    node_feats = np.asarray(node_feats, f32)
    edges = np.asarray(edges, f32)
    W_proj = np.ascontiguousarray(np.asarray(W_proj, f32))
    b_proj = np.asarray(b_proj, f32)
    a = np.asarray(a, f32)
    W_adj = np.asarray(W_adj, f32)
    b_adj = np.asarray(b_adj, f32)

    apm = np.zeros((C_OUT, H), f32)
    acm = np.zeros((C_OUT, H), f32)
    for h in range(H):
        apm[h * CPH:(h + 1) * CPH, h] = a[h, :CPH]
        acm[h * CPH:(h + 1) * CPH, h] = a[h, CPH:]
    ident = np.eye(128, dtype=f32)
    rhsi = np.zeros((17, H * QN), f32)
    for h in range(H):
        rhsi[1 + h, h * QN:(h + 1) * QN] = 1.0

    wadj_rot = [np.ascontiguousarray(np.roll(W_adj, -q * QN, axis=1)) for q in range(4)]

    in_maps = []
    for c in range(8):
        b, q = c // 4, c % 4
        et = np.ascontiguousarray(edges[b].T)
        nt = np.ascontiguousarray(node_feats[b].T)
        in_maps.append({
            "edgest": np.ascontiguousarray(np.roll(et, -q * QN, axis=1)),
            "wadjr": wadj_rot[q],
            "wchunk": np.ascontiguousarray(W_adj[:, q * QN:(q + 1) * QN]),
            "echunk": np.ascontiguousarray(et[:, q * QN:(q + 1) * QN]),
            "badjr": np.ascontiguousarray(np.roll(b_adj, -q * QN)[None, :]),
            "badji": np.ascontiguousarray(b_adj[None, q * QN:(q + 1) * QN]),
            "nft": np.ascontiguousarray(np.roll(nt, -q * QN, axis=1)),
            "nftc": np.ascontiguousarray(nt[:, q * QN:(q + 1) * QN]),
            "wproj": W_proj,
            "bprojr": np.ascontiguousarray(b_proj[None, :]),
            "apm": apm,
            "acm": acm,
            "ident": ident,
            "rhsi": rhsi,
        })
    return in_maps


LAST_RESULTS = None


def kernel(node_feats, edges, W_proj, b_proj, a, W_adj, b_adj, trace=False):
    global LAST_RESULTS
    from concourse.bass_utils import run_bass_kernel_spmd

    nc = _build_bass()
    in_maps = _prep_in_maps(node_feats, edges, W_proj, b_proj, a, W_adj, b_adj)
    res = run_bass_kernel_spmd(nc, in_maps, core_ids=list(range(8)), trace=trace)
    LAST_RESULTS = res
    out = np.empty((B, N, C_OUT), np.float32)
    for c in range(8):
        b, q = c // 4, c % 4
        out[b, q * QN:(q + 1) * QN, :] = res.results[c]["out"]
    return out


# revision 36
# speedup vs baseline: 1.1368x; 1.1368x over previous
# GAT layer kernel for Trainium2 (8 NeuronCores, SPMD).
#
# Reference computation (B=2, N=1024, C_IN=128, H=16, CPH=8):
#   e   = sigmoid(edges @ W_adj + b_adj);  e = (e + e^T)/2;  adj = e > 0.5
#   x   = node_feats @ W_proj + b_proj           -> [B,N,H,CPH]
#   lp  = sum_c x*a[:, :8];  lc = sum_c x*a[:, 8:]
#   L   = leaky_relu(lp[i,h] + lc[j,h], 0.2) masked by adj, softmax over j
#   out = einsum('bijh,bjhc->bihc', softmax, x)  -> [B,N,128]
#
# Sharding: core c handles batch b=c//4, parent rows i in chunk q=c%4 (256 rows).
#
# Key tricks:
#  * sigmoid is monotone => (sig(p)+sig(r))/2 > 0.5  <=>  p + r > 0.
#    So adj needs no sigmoid: adj[i,j] = (Zr[i,j]+Zr[j,i] > 0), Zr = edges@W_adj+b.
#  * Everything in the big [j, (h,i)] layout (j on partitions) so softmax sums
#    ride the PE einsum via a ones-column, and no on-device transposes of the
#    N^2 tensors are needed (host passes edges^T; both symmetrization terms
#    become natural-orientation matmuls).
#  * Softmax normalization applied to the *output* (divide by S after einsum).
#  * Per-core j-axis is rotated by q*256 so the SPMD program uses fixed slices.

import numpy as np

B, N, C_IN, H, CPH, C_OUT = 2, 1024, 128, 16, 8, 128
QN = N // 4          # rows per core
NKT = N // 128       # k tiles
NJT = N // 128       # j tiles
ALPHA = 0.2

_CACHE = {}
_DEBUG = False


def _build_bass():
    if "nc" in _CACHE:
        return _CACHE["nc"]
    from contextlib import ExitStack

    import concourse.mybir as mybir
    import concourse.tile as tile
    from concourse.bacc import Bacc

    fp32 = mybir.dt.float32
    bf16 = mybir.dt.bfloat16
    F = mybir.ActivationFunctionType
    ALU = mybir.AluOpType

    nc = Bacc()

    # DRAM I/O (per-core contents; j axis rotated by q*QN where noted)
    edgest_d = nc.dram_tensor("edgest", [N, N], fp32, kind="ExternalInput")   # edges[b].T, cols j-rotated
    wadjr_d = nc.dram_tensor("wadjr", [N, N], fp32, kind="ExternalInput")     # W_adj, cols j-rotated
    wchunk_d = nc.dram_tensor("wchunk", [N, QN], fp32, kind="ExternalInput")  # W_adj[:, chunk] (unrotated)
    echunk_d = nc.dram_tensor("echunk", [N, QN], fp32, kind="ExternalInput")  # edges[b].T[:, chunk]
    badjr_d = nc.dram_tensor("badjr", [1, N], fp32, kind="ExternalInput")     # b_adj, j-rotated
    badji_d = nc.dram_tensor("badji", [1, QN], fp32, kind="ExternalInput")    # b_adj[chunk]
    nft_d = nc.dram_tensor("nft", [C_IN, N], fp32, kind="ExternalInput")      # node_feats[b].T, cols j-rotated
    nftc_d = nc.dram_tensor("nftc", [C_IN, QN], fp32, kind="ExternalInput")   # node_feats[b,chunk].T
    wproj_d = nc.dram_tensor("wproj", [C_IN, C_OUT], fp32, kind="ExternalInput")
    bprojr_d = nc.dram_tensor("bprojr", [1, C_OUT], fp32, kind="ExternalInput")
    apm_d = nc.dram_tensor("apm", [C_OUT, H], fp32, kind="ExternalInput")     # block-diag a[:, :8]
    acm_d = nc.dram_tensor("acm", [C_OUT, H], fp32, kind="ExternalInput")     # block-diag a[:, 8:]
    ident_d = nc.dram_tensor("ident", [128, 128], fp32, kind="ExternalInput")
    lptmp_d = nc.dram_tensor("lptmp", [H * QN], fp32, kind="Internal")
    xmtmp_d = nc.dram_tensor("xmtmp", [C_OUT], fp32, kind="Internal")
    out_d = nc.dram_tensor("out", [QN, C_OUT], fp32, kind="ExternalOutput")
    if _DEBUG:
        dbg = {
            "d_m01": nc.dram_tensor("d_m01", [128, QN], fp32, kind="ExternalOutput"),
            "d_u": nc.dram_tensor("d_u", [128, H * QN], fp32, kind="ExternalOutput"),
            "d_lr": nc.dram_tensor("d_lr", [128, H * QN], fp32, kind="ExternalOutput"),
            "d_rhs17": nc.dram_tensor("d_rhs17", [17, H * QN], fp32, kind="ExternalOutput"),
            "d_lcones": nc.dram_tensor("d_lcones", [17, N], fp32, kind="ExternalOutput"),
            "d_xe0": nc.dram_tensor("d_xe0", [128, H * (CPH + 1)], fp32, kind="ExternalOutput"),
            "d_outts": nc.dram_tensor("d_outts", [2, 128, 512], fp32, kind="ExternalOutput"),
            "d_sis": nc.dram_tensor("d_sis", [2, 128, H], fp32, kind="ExternalOutput"),
        }

    with tile.TileContext(nc) as tc, ExitStack() as ctx:
        const = ctx.enter_context(tc.tile_pool(name="const", bufs=1))
        work = ctx.enter_context(tc.tile_pool(name="work", bufs=2))
        fin = ctx.enter_context(tc.tile_pool(name="fin", bufs=1))
        psq = ctx.enter_context(tc.tile_pool(name="psq", bufs=3, space="PSUM"))
        pse = ctx.enter_context(tc.tile_pool(name="pse", bufs=1, space="PSUM"))

        # ---------------- residents ----------------
        edt = const.tile([128, NKT, N], fp32, name="edt")      # edges^T (rot), [p, kt, j]
        wad = const.tile([128, NKT, N], fp32, name="wad")      # W_adj (rot cols)
        wch = const.tile([128, NKT, QN], fp32, name="wch")     # W_adj[:, chunk]
        ech = const.tile([128, NKT, QN], fp32, name="ech")     # edges^T[:, chunk]
        for kt in range(NKT):
            nc.sync.dma_start(edt[:, kt, :], edgest_d[kt * 128:(kt + 1) * 128, :])
            nc.sync.dma_start(wad[:, kt, :], wadjr_d[kt * 128:(kt + 1) * 128, :])
            nc.sync.dma_start(wch[:, kt, :], wchunk_d[kt * 128:(kt + 1) * 128, :])
            nc.sync.dma_start(ech[:, kt, :], echunk_d[kt * 128:(kt + 1) * 128, :])

        nft_s = const.tile([128, N], fp32, name="nft_s")
        nc.sync.dma_start(nft_s, nft_d[:, :])
        nftc_s = const.tile([128, QN], fp32, name="nftc_s")
        nc.sync.dma_start(nftc_s, nftc_d[:, :])
        wproj_s = const.tile([128, C_OUT], fp32, name="wproj_s")
        nc.sync.dma_start(wproj_s, wproj_d[:, :])
        bprojr_s = const.tile([1, C_OUT], fp32, name="bprojr_s")
        nc.sync.dma_start(bprojr_s, bprojr_d[:, :])
        apm_s = const.tile([128, H], fp32, name="apm_s")
        nc.sync.dma_start(apm_s, apm_d[:, :])
        acm_s = const.tile([128, H], fp32, name="acm_s")
        nc.sync.dma_start(acm_s, acm_d[:, :])
        badjr_s = const.tile([1, N], fp32, name="badjr_s")
        nc.sync.dma_start(badjr_s, badjr_d[:, :])
        badji_s = const.tile([1, QN], fp32, name="badji_s")
        nc.sync.dma_start(badji_s, badji_d[:, :])
        ident_s = const.tile([128, 128], fp32, name="ident_s")
        nc.sync.dma_start(ident_s, ident_d[:, :])

        ones_row = const.tile([1, N], fp32, name="ones_row")
        nc.vector.memset(ones_row, 1.0)

        # ---------------- phase 0: projections ----------------
        # x^T [hc, n] for lc (rotated j space), full N
        xt_s = const.tile([128, N], fp32, name="xt_s")
        for half in range(2):
            ps = psq.tile([128, 512], fp32, tag="q")
            nc.tensor.matmul(ps, wproj_s, nft_s[:, half * 512:(half + 1) * 512],
                             start=True, stop=False)
            nc.tensor.matmul(ps, bprojr_s, ones_row[:, :512], start=False, stop=True)
            nc.scalar.copy(xt_s[:, half * 512:(half + 1) * 512], ps)

        # x^T for the i-chunk (unrotated) -> lp
        xtc_s = const.tile([128, QN], fp32, name="xtc_s")
        psc = psq.tile([128, 512], fp32, tag="q")
        nc.tensor.matmul(psc[:, :QN], wproj_s, nftc_s, start=True, stop=False)
        nc.tensor.matmul(psc[:, :QN], bprojr_s, ones_row[:, :QN], start=False, stop=True)
        nc.scalar.copy(xtc_s, psc[:, :QN])

        # lp [16, 256] -> lp_flat [1, 4096] (h,i) via DRAM bounce, then
        # broadcast across partitions with K=1 matmuls -> LPB [128, (h,i)]
        lp_ps = psq.tile([128, 512], fp32, tag="q")
        nc.tensor.matmul(lp_ps[:H, :QN], apm_s, xtc_s, start=True, stop=True)
        lp_s = const.tile([H, QN], fp32, name="lp_s")
        nc.vector.tensor_copy(lp_s, lp_ps[:H, :QN])
        nc.sync.dma_start(lptmp_d.rearrange("(h i) -> h i", h=H), lp_s)
        lpflat_s = const.tile([1, H * QN], fp32, name="lpflat_s")
        nc.sync.dma_start(lpflat_s, lptmp_d[None, :])
        lpb = const.tile([128, H, QN], fp32, name="lpb")
        lpbf = lpb.rearrange("p h i -> p (h i)")
        for c8 in range(8):
            ps8 = psq.tile([128, 512], fp32, tag="q")
            nc.tensor.matmul(ps8, ones_row[:, :128],
                             lpflat_s[:, c8 * 512:(c8 + 1) * 512],
                             start=True, stop=True)
            if c8 % 2:
                nc.scalar.copy(lpbf[:, c8 * 512:(c8 + 1) * 512], ps8)
            else:
                nc.vector.tensor_copy(lpbf[:, c8 * 512:(c8 + 1) * 512], ps8)

        # lc in [j, h] layout, per j-tile
        lcn = []
        for jt in range(NJT):
            psl = psq.tile([128, 512], fp32, tag="q")
            nc.tensor.matmul(psl[:, :H], xt_s[:, jt * 128:(jt + 1) * 128], acm_s,
                             start=True, stop=True)
            lcj = const.tile([128, H], fp32, name=f"lcn{jt}")
            nc.vector.tensor_copy(lcj, psl[:, :H])
            lcn.append(lcj)

        # column means of x (uniform-attention fallback for all-masked rows):
        # reduce x^T over n, bounce through DRAM to a row, broadcast via PE.
        xsum_t = const.tile([128, 1], fp32, name="xsum_t")
        nc.vector.tensor_reduce(xsum_t, xt_s, mybir.AxisListType.X, ALU.add)
        nc.sync.dma_start(xmtmp_d.rearrange("(p o) -> p o", o=1), xsum_t)
        xmr_s = const.tile([1, C_OUT], fp32, name="xmr_s")
        nc.sync.dma_start(xmr_s, xmtmp_d[None, :])
        xm_ps = psq.tile([128, 512], fp32, tag="q")
        nc.tensor.matmul(xm_ps[:, :C_OUT], ones_row[:, :128], xmr_s,
                         start=True, stop=True)
        xmb = const.tile([128, H, CPH], fp32, name="xmb")
        nc.vector.tensor_scalar_mul(
            xmb.rearrange("p h c -> p (h c)"), xm_ps[:, :C_OUT], 1.0 / N)

        # x~ (einsum stationary): per j-tile [128j, 16h, 9] bf16, col 8 = 1.0
        xe = []
        for jt in range(NJT):
            ps = psq.tile([128, 512], fp32, tag="q")
            nc.tensor.matmul(ps[:, :128], nft_s[:, jt * 128:(jt + 1) * 128], wproj_s,
                             start=True, stop=False)
            nc.tensor.matmul(ps[:, :128], ones_row[:, :128], bprojr_s, start=False, stop=True)
            xej = const.tile([128, H, CPH + 1], bf16, name=f"xe{jt}")
            nc.vector.tensor_copy(
                xej[:, :, 0:CPH],
                ps[:, :128].rearrange("p (h c) -> p h c", c=CPH),
            )
            nc.vector.memset(xej[:, :, CPH:CPH + 1], 1.0)
            xe.append(xej)

        # einsum accumulators: head pair p -> tile t=p//4, partitions
        # 32r..32r+18 with r=p%4. Rows 32r+9e+c hold head 2p+e (valid free
        # half e*256..); row 32r+9e+8 is that head's softmax denominator S.
        ab = [pse.tile([128, 512], fp32, name=f"ab{t}") for t in range(2)]

        # ---------------- main loop over j tiles ----------------
        for jt in range(NJT):
            # adjacency: Q[j,i] = Zr[j,i] + Zr[i,j] + b_adj[i] + b_adj[j]
            qps = psq.tile([128, 512], fp32, tag="q")
            q_ap = qps[:, :QN]
            for kt in range(NKT):
                nc.tensor.matmul(q_ap, edt[:, kt, jt * 128:(jt + 1) * 128],
                                 wch[:, kt, :], start=(kt == 0), stop=False)
                nc.tensor.matmul(q_ap, wad[:, kt, jt * 128:(jt + 1) * 128],
                                 ech[:, kt, :], start=False, stop=False)
            nc.tensor.matmul(q_ap, badjr_s[:, jt * 128:(jt + 1) * 128],
                             ones_row[:, :QN], start=False, stop=False)
            nc.tensor.matmul(q_ap, ones_row[:, :128], badji_s, start=False, stop=True)
            m01 = work.tile([128, QN], bf16, tag="m01")
            nc.vector.tensor_scalar(m01, q_ap, 0.0, None, ALU.is_gt)
            if _DEBUG and jt == 0:
                dm = fin.tile([128, QN], fp32, name="dm")
                nc.vector.tensor_copy(dm, m01)
                nc.sync.dma_start(dbg["d_m01"][:, :], dm)

            # logits Z[j,(h,i)] = lp[i,h] (broadcast tile) + lc[j,h] (per-
            # partition scalar), on DVE; then lrelu+exp on ACT; mask on GPSIMD.
            z_s = work.tile([128, H, QN], fp32, tag="z")
            for h in range(H):
                nc.vector.tensor_scalar(z_s[:, h, :], lpb[:, h, :],
                                        lcn[jt][:, h:h + 1], None, ALU.add)
            zf = z_s.rearrange("p h i -> p (h i)")
            nc.scalar.activation(zf, zf, F.Prelu, alpha=ALPHA)
            u = work.tile([128, H, QN], bf16, tag="u")
            uflat = u.rearrange("p h i -> p (h i)")
            nc.scalar.activation(uflat, zf, F.Exp)
            nc.gpsimd.tensor_tensor(
                u, u, m01[:, None, :].to_broadcast((128, H, QN)), ALU.mult)
            if _DEBUG and jt == 0:
                nc.sync.dma_start(dbg["d_lr"][:, :], zf)
                du = fin.tile([128, H * QN], fp32, name="du")
                nc.vector.tensor_copy(du, uflat)
                nc.sync.dma_start(dbg["d_u"][:, :], du)
                nc.sync.dma_start(dbg["d_rhs17"][:, :], rhs17)
                nc.sync.dma_start(dbg["d_lcones"][:, :], lcones)
                dx = fin.tile([128, H * (CPH + 1)], fp32, name="dx")
                nc.vector.tensor_copy(dx, xe[0].rearrange("p h c -> p (h c)"))
                nc.sync.dma_start(dbg["d_xe0"][:, :], dx)

            # einsum: out^T[(h,c),i] += x~_h^T @ U_h  (col 8 accumulates S)
            # einsum, two heads per matmul: lhsT = [x~_2p | x~_2p+1] (M=18),
            # rhs = U cols for both heads (N=512). Off-diagonal cross blocks
            # land in unused psum cells and are ignored at copy-out.
            for p in range(8):
                t, r = p // 4, p % 4
                nc.tensor.matmul(
                    ab[t][32 * r:32 * r + 18, :],
                    xe[jt][:, 2 * p:2 * p + 2, :],
                    uflat[:, p * 512:(p + 1) * 512],
                    start=(jt == 0), stop=(jt == NJT - 1),
                    tile_position=(0, 32 * r), skip_group_check=True)

        # ---------------- finalize: transpose, normalize, store ----------------
        outts = [fin.tile([128, 512], fp32, name=f"outts{t}") for t in range(2)]
        for t in range(2):
            nc.vector.memset(outts[t], 0.0)
            for r in range(4):
                if (t + r) % 2:
                    nc.scalar.copy(outts[t][32 * r:32 * r + 18, :],
                                   ab[t][32 * r:32 * r + 18, :])
                else:
                    nc.vector.tensor_copy(outts[t][32 * r:32 * r + 18, :],
                                          ab[t][32 * r:32 * r + 18, :])

        o_s = [fin.tile([128, H, CPH], fp32, name=f"os{ih}") for ih in range(2)]
        s_is = [fin.tile([128, H], fp32, name=f"sis{ih}") for ih in range(2)]
        for t in range(2):
            for gb in range(2):
                for ih in range(2):
                    pst = psq.tile([128, 512], fp32, tag="q")
                    nc.tensor.transpose(
                        pst[:, :128],
                        outts[t][:, gb * 256 + ih * 128: gb * 256 + (ih + 1) * 128],
                        ident_s)
                    # psT col 32r + 9e + c holds head 2(4t+r)+e (valid e == gb)
                    p3 = pst[:, :128].rearrange("p (r c) -> p r c", c=32)
                    o4 = o_s[ih].rearrange("p (hh e) c -> p hh e c", e=2)
                    s4 = s_is[ih].rearrange("p (hh e) -> p hh e", e=2)
                    nc.vector.tensor_copy(
                        o4[:, 4 * t:4 * t + 4, gb, :],
                        p3[:, :, 9 * gb:9 * gb + CPH])
                    nc.vector.tensor_copy(
                        s4[:, 4 * t:4 * t + 4, gb, None],
                        p3[:, :, 9 * gb + CPH:9 * gb + CPH + 1])
        if _DEBUG:
            for t in range(2):
                nc.sync.dma_start(dbg["d_outts"][t, :, :], outts[t])
            for ih in range(2):
                nc.sync.dma_start(dbg["d_sis"][ih, :, :], s_is[ih])
        for ih in range(2):
            # guard against S=0 (all-masked row): reference softmax degrades to
            # uniform 1/N over all j there; patch those (i,h) with column means.
            s0 = fin.tile([128, H, CPH], mybir.dt.uint8, name=f"s0{ih}")
            nc.vector.tensor_scalar(
                s0, s_is[ih][:, :, None].to_broadcast((128, H, CPH)),
                0.0, None, ALU.is_equal)
            r_is = fin.tile([128, H], fp32, name=f"ris{ih}")
            nc.vector.tensor_scalar_max(r_is, s_is[ih], 1e-30)
            nc.vector.reciprocal(r_is, r_is)
            nc.vector.tensor_tensor(
                o_s[ih], o_s[ih], r_is[:, :, None].to_broadcast((128, H, CPH)),
                ALU.mult)
            nc.vector.copy_predicated(o_s[ih], s0, xmb)
            nc.sync.dma_start(
                out_d[ih * 128:(ih + 1) * 128, :],
                o_s[ih].rearrange("p h c -> p (h c)"))

    nc.finalize()
    _CACHE["nc"] = nc
    return nc


def _prep_in_maps(node_feats, edges, W_proj, b_proj, a, W_adj, b_adj):
    f32 = np.float32
    node_feats = np.asarray(node_feats, f32)
    edges = np.asarray(edges, f32)
    W_proj = np.ascontiguousarray(np.asarray(W_proj, f32))
    b_proj = np.asarray(b_proj, f32)
    a = np.asarray(a, f32)
    W_adj = np.asarray(W_adj, f32)
    b_adj = np.asarray(b_adj, f32)

    apm = np.zeros((C_OUT, H), f32)
    acm = np.zeros((C_OUT, H), f32)
    for h in range(H):
        apm[h * CPH:(h + 1) * CPH, h] = a[h, :CPH]
        acm[h * CPH:(h + 1) * CPH, h] = a[h, CPH:]
    ident = np.eye(128, dtype=f32)

    wadj_rot = [np.ascontiguousarray(np.roll(W_adj, -q * QN, axis=1)) for q in range(4)]

    in_maps = []
    for c in range(8):
        b, q = c // 4, c % 4
        et = np.ascontiguousarray(edges[b].T)
        nt = np.ascontiguousarray(node_feats[b].T)
        in_maps.append({
            "edgest": np.ascontiguousarray(np.roll(et, -q * QN, axis=1)),
            "wadjr": wadj_rot[q],
            "wchunk": np.ascontiguousarray(W_adj[:, q * QN:(q + 1) * QN]),
            "echunk": np.ascontiguousarray(et[:, q * QN:(q + 1) * QN]),
            "badjr": np.ascontiguousarray(np.roll(b_adj, -q * QN)[None, :]),
            "badji": np.ascontiguousarray(b_adj[None, q * QN:(q + 1) * QN]),
            "nft": np.ascontiguousarray(np.roll(nt, -q * QN, axis=1)),
            "nftc": np.ascontiguousarray(nt[:, q * QN:(q + 1) * QN]),
            "wproj": W_proj,
            "bprojr": np.ascontiguousarray(b_proj[None, :]),
            "apm": apm,
            "acm": acm,
            "ident": ident,
        })
    return in_maps


LAST_RESULTS = None


def kernel(node_feats, edges, W_proj, b_proj, a, W_adj, b_adj, trace=False):
    global LAST_RESULTS
    from concourse.bass_utils import run_bass_kernel_spmd

    nc = _build_bass()
    in_maps = _prep_in_maps(node_feats, edges, W_proj, b_proj, a, W_adj, b_adj)
    res = run_bass_kernel_spmd(nc, in_maps, core_ids=list(range(8)), trace=trace)
    LAST_RESULTS = res
    out = np.empty((B, N, C_OUT), np.float32)
    for c in range(8):
        b, q = c // 4, c % 4
        out[b, q * QN:(q + 1) * QN, :] = res.results[c]["out"]
    return out
